# revision 49
# baseline (speedup 1.0000x reference)
"""ChebNet (K=3, L=2) forward on 8 Trainium2 NeuronCores.

Node-sharded SPMD, compiled per graph instance. Each core owns 6250 dst rows,
LPT-permuted into 104 windows of 64 columns so window edge counts are nearly
equal; the shared program uses max-over-cores tile counts per (chunk, half,
window) group (~10% slot padding vs 60% for a fixed layout).

Gather tables are bf16 with two node rows packed per 256B table row (the
dma_gather minimum element); edge slots are grouped by (window, src parity)
so each 128-slot tile reads one half of its gathered elements as the PE
lhsT — no separate parity masks. SpMM = one dma_gather per (chunk, parity)
+ bf16 PE matmuls against 0/1 masks generated in bf16. norm[src] is folded
into the gather tables, norm[dst] into a per-column multiply of the dense
accumulation, which commutes through the weight matmul:

  P1 = A(Nx); T1 = -N P1; table2 = -N^2 P1; P2 = A table2; T2 = -2N P2 - T0
  out^T = (W0-W2)^T T0^T + N o [(-W1)^T P1^T + (-2 W2)^T P2^T]; h = relu(out+b)

Full node tables are rebuilt between SpMMs with AllGather. The table rows are
laid out [chunk-group][core][rows] so each AllGather is split into NG pieces,
each fired as soon as its chunk-group's rows are evacuated — the collective
pipeline overlaps the producing SpMM's remaining compute.
"""
import math
import numpy as np

N, E, D, K, L = 50000, 800000, 64, 3, 2
NCORES = 8
NSH = N // NCORES              # 6250 dst rows per core
MWIN = 64                      # mask window: dst cols per mask tile
CHW = 8                        # windows per chunk
CCOLS = MWIN * CHW             # 512 cols per chunk (one PSUM bank)
NCH = 13                       # chunks per core
NCOLS = NCH * CCOLS            # 6656 padded cols per core
NW = NCH * CHW                 # 104 windows per core
ROWS_PAD = NCOLS               # padded rows per core block in the table
TBL_ROWS = NCORES * ROWS_PAD   # 53248
TBLP = TBL_ROWS // 2           # 26624 paired bf16 rows (< 32768: int16)
import os as _os
GMAX_T = int(_os.environ.get("K_GMAX_T", "8"))  # max tiles per dma_gather
# (>8 tiles = >1024 idxs per call overruns the SWDGE ring and hangs the HW)
NQUEUES = int(_os.environ.get("K_NQUEUES", "4"))
GBUFS = int(_os.environ.get("K_GBUFS", "2"))
DENSE_CONTIG = _os.environ.get("K_DENSE_CONTIG", "0") == "1"

# AllGather pipeline: chunk-groups; piece g fires after its chunks evacuate
CGRP = [5, 4, 4]
NG = len(CGRP)
CSTART = [sum(CGRP[:g]) for g in range(NG)]          # first chunk of group
CEND = [sum(CGRP[:g + 1]) for g in range(NG)]        # one-past-last chunk
GRP_OF = sum(([g] * CGRP[g] for g in range(NG)), [])
GOFF = [NCORES * CCOLS * CSTART[g] for g in range(NG)]  # table row offset

_CACHE = {}


# ---------------------------------------------------------------------------
# Workaround for this walrus build: any instruction carrying >1 sync wait is
# rejected ("Too many sync wait commands"). Hoist extras onto 1-wait NoOps on
# the same engine (per-engine program order preserves semantics).
_ws_counter = [0]


def _split_multiwaits(nc):
    import concourse.mybir as mybir
    n_split = 0
    for fn in nc.m.functions:
        for bb in fn.blocks:
            new_list = []
            changed = False
            for inst in bb.instructions:
                si = inst.sync_info
                waits = list(si.on_wait) if si is not None else []
                if len(waits) > 1:
                    changed = True
                    for w in waits[:-1]:
                        _ws_counter[0] += 1
                        nop = mybir.InstNoOp(
                            name=f"waitsplit-{_ws_counter[0]}",
                            ins=[], outs=[],
                            sync_info=mybir.SyncInfo(on_wait=[w], on_update=[]),
                        )
                        nop.engine = inst.engine
                        nc.register_instruction(nop, overwrite=True)
                        new_list.append(nop)
                        n_split += 1
                    si.on_wait = waits[-1:]
                new_list.append(inst)
            if changed:
                bb.instructions[:] = new_list
    return n_split


def _finalize_with_split(nc):
    import concourse.bass as _bass
    nc.compile()           # Bacc passes (incl. library-load insertion)
    _split_multiwaits(nc)  # after replace_nops_with_events, before freeze
    _bass.Bass.finalize(nc)


def _build_runner(nc, n_cores):
    """SPMD runner over the axon PJRT backend (keeps the jitted executable
    and device-resident inputs so repeat calls can be timed)."""
    import jax
    from jax.sharding import Mesh, PartitionSpec
    from jax.experimental.shard_map import shard_map
    import concourse.mybir as mybir
    from concourse.bass2jax import (
        _bass_exec_p, install_neuronx_cc_hook, partition_id_tensor)

    install_neuronx_cc_hook()
    partition_name = nc.partition_id_tensor.name if nc.partition_id_tensor else None

    in_names, out_names, out_avals, zero_outs = [], [], [], []
    for alloc in nc.m.functions[0].allocations:
        if not isinstance(alloc, mybir.MemoryLocationSet):
            continue
        name = alloc.memorylocations[0].name
        if alloc.kind == "ExternalInput":
            if name != partition_name:
                in_names.append(name)
        elif alloc.kind == "ExternalOutput":
            shape = tuple(alloc.tensor_shape)
            dtype = mybir.dt.np(alloc.dtype)
            out_names.append(name)
            out_avals.append(jax.core.ShapedArray(shape, dtype))
            zero_outs.append(np.zeros(shape, dtype))
    n_params = len(in_names)
    all_in_names = list(in_names) + list(out_names)
    if partition_name is not None:
        all_in_names.append(partition_name)

    def _body(*args):
        operands = list(args)
        if partition_name is not None:
            operands.append(partition_id_tensor())
        outs = _bass_exec_p.bind(
            *operands,
            out_avals=tuple(out_avals),
            in_names=tuple(all_in_names),
            out_names=tuple(out_names),
            lowering_input_output_aliases=(),
            sim_require_finite=True,
            sim_require_nnan=True,
            nc=nc,
        )
        return tuple(outs)

    devices = jax.devices()[:n_cores]
    mesh = Mesh(np.asarray(devices), ("core",))
    in_specs = (PartitionSpec("core"),) * (n_params + len(out_names))
    out_specs = (PartitionSpec("core"),) * len(out_names)
    sharded = jax.jit(
        shard_map(_body, mesh=mesh, in_specs=in_specs, out_specs=out_specs,
                  check_rep=False),
        keep_unused=True,
    )

    def run(in_maps, iters=1):
        import time as _time
        per_core = [[np.asarray(m[name]) for name in in_names] for m in in_maps]
        concat_in = [
            np.concatenate([per_core[c][i] for c in range(n_cores)], axis=0)
            for i in range(n_params)
        ]
        concat_zeros = [
            np.zeros((n_cores * z.shape[0], *z.shape[1:]), z.dtype)
            for z in zero_outs
        ]
        sharding = jax.sharding.NamedSharding(mesh, PartitionSpec("core"))
        dev_in = [jax.device_put(a, sharding) for a in concat_in + concat_zeros]
        out = sharded(*dev_in)
        jax.block_until_ready(out)
        times = []
        for _ in range(iters):
            t0 = _time.perf_counter()
            out = sharded(*dev_in)
            jax.block_until_ready(out)
            times.append(_time.perf_counter() - t0)
        results = [
            {name: np.asarray(out[i]).reshape(n_cores, *out_avals[i].shape)[c]
             for i, name in enumerate(out_names)}
            for c in range(n_cores)
        ]
        return results, times

    return run


def _lpt_windows(deg_local):
    """Assign each of the core's nodes to one of NW windows (<=64 nodes each)
    balancing total degree (LPT greedy). Returns pos[node] in [0, NCOLS)."""
    import heapq
    n = deg_local.shape[0]
    order = np.argsort(-deg_local, kind="stable")
    heap = [(0.0, w) for w in range(NW)]
    heapq.heapify(heap)
    counts = np.zeros(NW, dtype=np.int64)
    pos = np.empty(n, dtype=np.int64)
    for node in order:
        s, w = heapq.heappop(heap)
        pos[node] = (w // CHW) * CCOLS + (w % CHW) * MWIN + counts[w]
        counts[w] += 1
        s += deg_local[node]
        if counts[w] < MWIN:
            heapq.heappush(heap, (s, w))
    return pos


GOFF2 = [NCORES * CSTART[g] * (CCOLS // 2) for g in range(NG)]


def _tbl_row(core, pos):
    """(pair row, parity) for (core, permuted col pos): group-major layout,
    pairing positions (128k+q, 128k+64+q) into one 128-wide bf16 table row."""
    c = pos // CCOLS
    g = np.asarray(GRP_OF, dtype=np.int64)[c]
    goff = np.asarray(GOFF2, dtype=np.int64)[g]
    cstart = np.asarray(CSTART, dtype=np.int64)[g]
    p_loc = pos - cstart * CCOLS          # position within the group's chunks
    kb = p_loc // 128
    q = p_loc % 128
    pair = goff + core * (np.asarray(CGRP, np.int64)[g] * (CCOLS // 2)) \
        + kb * 64 + q % 64
    return pair, q // 64


def _host_prep(features, src, dst, W, b, pw, pb):
    import concourse.mybir as mybir
    bf16 = mybir.dt.np(mybir.dt.bfloat16)

    src = np.asarray(src).astype(np.int64)
    dst = np.asarray(dst).astype(np.int64)
    features = np.asarray(features, dtype=np.float32)
    W = np.asarray(W, dtype=np.float32)
    b = np.asarray(b, dtype=np.float32)
    pw = np.asarray(pw, dtype=np.float32).reshape(D, 1)
    pb = np.asarray(pb, dtype=np.float32).reshape(1)

    deg = np.bincount(dst, minlength=N).astype(np.float32)
    norm = np.clip(deg, 1.0, None) ** -0.5

    core_of_dst = dst // NSH
    pos_local = np.empty(N, dtype=np.int64)   # permuted col within core
    for i in range(NCORES):
        nodes = np.arange(i * NSH, (i + 1) * NSH)
        pos_local[nodes] = _lpt_windows(deg[nodes])
    gpair, gpar = _tbl_row(np.arange(N) // NSH, pos_local)  # paired table row

    # per-edge quantities
    e_core = core_of_dst
    e_pos = pos_local[dst]                   # dst col within owner core
    e_win = e_pos // MWIN                    # global window 0..103
    e_dcol = (e_pos % MWIN).astype(np.float32)
    e_srow = gpair[src]                      # src pair row in packed table
    e_half = gpar[src]                       # src parity within the pair

    # group counts per (core, win, parity) -> shared tile counts k = max
    gk = e_core * (NW * 2) + e_win * 2 + e_half
    cnt = np.bincount(gk, minlength=NCORES * NW * 2).reshape(NCORES, NW, 2)
    kk = -(-cnt.max(axis=0) // 128)          # [NW, 2] tiles per (win, half)
    kk = np.maximum(kk, 1)

    # slot/tile layout shared by all cores.
    # chunk c: call h=0 -> windows 0..7 tiles, then call h=1.
    kc = kk.reshape(NCH, CHW, 2)
    call_tiles = kc.sum(axis=1)              # [NCH, 2] tiles per call
    call_slots = call_tiles * 128
    ntile_chunk = call_tiles.sum(axis=1)     # [NCH]
    NTILE_TOT = int(ntile_chunk.sum())
    SLOTS_TOT = NTILE_TOT * 128
    # tile base within chunk for (h, w)
    tile_base = np.zeros((NCH, 2, CHW), dtype=np.int64)
    for c in range(NCH):
        t = 0
        for h in (0, 1):
            for w in range(CHW):
                tile_base[c, h, w] = t
                t += kc[c, w, h]
    chunk_tile_off = np.concatenate([[0], np.cumsum(ntile_chunk)])[:-1]
    chunk_slot_off = chunk_tile_off * 128
    call_slot_off = np.zeros((NCH, 2), dtype=np.int64)
    for c in range(NCH):
        call_slot_off[c, 0] = chunk_slot_off[c]
        call_slot_off[c, 1] = chunk_slot_off[c] + call_slots[c, 0]

    meta = {
        "kc": kc, "call_tiles": call_tiles, "call_slots": call_slots,
        "ntile_chunk": ntile_chunk, "NTILE_TOT": NTILE_TOT,
        "SLOTS_TOT": SLOTS_TOT, "tile_base": tile_base,
        "chunk_tile_off": chunk_tile_off,
    }

    # shared scaled feature table (bf16, two node rows packed per table row)
    feat_scaled = np.zeros((TBLP, 2, D), dtype=bf16)
    feat_scaled[gpair, gpar] = (features * norm[:, None]).astype(bf16)
    feat_scaled = feat_scaled.reshape(TBLP, 2 * D)

    Wflat = np.zeros((D, L * 3 * D), dtype=np.float32)
    for l in range(L):
        for t, Wt in enumerate((W[l, 0] - W[l, 2], -W[l, 1], -2.0 * W[l, 2])):
            Wflat[:, (l * 3 + t) * D:(l * 3 + t + 1) * D] = Wt
    Wflat = Wflat.astype(bf16)

    iota = np.tile(np.arange(MWIN, dtype=np.float32)[None, :],
                   (128, 1)).astype(bf16)

    in_maps = []
    perms = []
    for i in range(NCORES):
        sel = e_core == i
        s_srow = e_srow[sel]
        s_half = e_half[sel]
        s_win = e_win[sel]
        s_dcol = e_dcol[sel]

        # rank within (win, half) group
        key = s_win * 2 + s_half
        order = np.argsort(key, kind="stable")
        ks = key[order]
        grp_start = np.searchsorted(ks, np.arange(NW * 2), side="left")
        rank = np.arange(ks.size) - grp_start[ks]

        ww = ks // 2
        hh = ks % 2
        c_ = ww // CHW
        wl = ww % CHW
        mt = chunk_tile_off[c_] + tile_base[c_, hh, wl] + rank // 128
        slot = mt * 128 + rank % 128

        idx_slots = np.zeros(SLOTS_TOT, dtype=np.int16)
        dcol_slots = np.full(SLOTS_TOT, -1.0, dtype=np.float32)
        idx_slots[slot] = s_srow[order].astype(np.int16)
        dcol_slots[slot] = s_dcol[order]

        # idx wrap per gather call: global 1024-slot windows (call k covers
        # tiles [8k, 8k+8)); within a call, slot i -> [i%16, i//16]
        idx_arr = np.zeros((16, SLOTS_TOT // 16), dtype=np.int16)
        for k in range(-(-NTILE_TOT // GMAX_T)):
            s0 = k * GMAX_T * 128
            n_ = min(GMAX_T * 128, SLOTS_TOT - s0)
            wrap = idx_slots[s0:s0 + n_].reshape(n_ // 16, 16).T
            idx_arr[:, s0 // 16:(s0 + n_) // 16] = wrap
        idx_arr = np.tile(idx_arr, (8, 1))

        dcol = dcol_slots.reshape(NTILE_TOT, 128).T.astype(bf16)

        # local node data in permuted order
        pos = pos_local[i * NSH:(i + 1) * NSH]
        perms.append(pos)
        nloc = norm[i * NSH:(i + 1) * NSH]
        f0T = np.zeros((D, NCOLS), dtype=np.float32)
        f0T[:, pos] = features[i * NSH:(i + 1) * NSH].T
        f0T = f0T.astype(bf16)
        normB_row = np.zeros(NCOLS, dtype=np.float32)
        normB_row[pos] = nloc
        normB = np.tile(normB_row[None, :], (D, 1))
        nsq_m2 = np.zeros((128, NCOLS // 128), dtype=np.float32)
        nsq_p1 = np.zeros((128, NCOLS // 128), dtype=np.float32)
        nrm_cols = np.zeros(NCOLS, dtype=np.float32)
        nrm_cols[pos] = nloc
        nsq_m2[:, :] = (-(nrm_cols ** 2)).reshape(NCOLS // 128, 128).T
        nsq_p1[:, :] = nrm_cols.reshape(NCOLS // 128, 128).T

        in_maps.append({
            "feat_tbl": feat_scaled,
            "f0T": f0T,
            "idx_all": idx_arr,
            "dcol": dcol,
            "iota": iota,
            "normB": normB,
            "nsq_m2": nsq_m2,
            "nsq_p1": nsq_p1,
            "Wflat": Wflat,
            "bvec": b.T.copy(),
            "pwv": pw.astype(bf16),
            "pbv": pb.reshape(1, 1),
        })
    return in_maps, meta, perms


def _build_nc(meta, repeat=1, mode="full"):
    import concourse.bacc as bacc
    import concourse.mybir as mybir
    import concourse.tile as tile
    from concourse.masks import make_identity
    f32 = mybir.dt.float32
    bf16 = mybir.dt.bfloat16
    i16 = mybir.dt.int16

    kc = meta["kc"]
    call_tiles = meta["call_tiles"]
    call_slots = meta["call_slots"]
    ntile_chunk = meta["ntile_chunk"]
    NTILE_TOT = meta["NTILE_TOT"]
    SLOTS_TOT = meta["SLOTS_TOT"]
    tile_base = meta["tile_base"]
    chunk_tile_off = meta["chunk_tile_off"]

    nc = bacc.Bacc("TRN2", num_swdge_queues=NQUEUES)
    feat_tbl = nc.declare_dram_parameter("feat_tbl", [TBLP, 2 * D], bf16, isOutput=False)
    f0T_in = nc.declare_dram_parameter("f0T", [D, NCOLS], bf16, isOutput=False)
    idx_in = nc.declare_dram_parameter("idx_all", [128, SLOTS_TOT // 16], i16, isOutput=False)
    dcol_in = nc.declare_dram_parameter("dcol", [128, NTILE_TOT], bf16, isOutput=False)
    iota_in = nc.declare_dram_parameter("iota", [128, MWIN], bf16, isOutput=False)
    normB_in = nc.declare_dram_parameter("normB", [D, NCOLS], f32, isOutput=False)
    nsqm2_in = nc.declare_dram_parameter("nsq_m2", [128, NCOLS // 128], f32, isOutput=False)
    nsqp1_in = nc.declare_dram_parameter("nsq_p1", [128, NCOLS // 128], f32, isOutput=False)
    W_in = nc.declare_dram_parameter("Wflat", [D, L * 3 * D], bf16, isOutput=False)
    b_in = nc.declare_dram_parameter("bvec", [D, L], f32, isOutput=False)
    pw_in = nc.declare_dram_parameter("pwv", [D, 1], bf16, isOutput=False)
    pb_in = nc.declare_dram_parameter("pbv", [1, 1], f32, isOutput=False)
    y_out = nc.declare_dram_parameter("y", [NCOLS, 1], f32, isOutput=True)

    ag = {}
    for nm in ("t2", "t3", "t4"):
        agins = [nc.dram_tensor(f"agin_{nm}_{g}",
                                [CGRP[g] * (CCOLS // 2), 2 * D], bf16)
                 for g in range(NG)]
        tbl = nc.dram_tensor(f"tbl_{nm}", [TBLP, 2 * D], bf16,
                             addr_space="Shared")
        ag[nm] = (agins, tbl)

    def bfview(ap):
        """View the high half-words of an f32 AP as a stride-2 bf16 AP."""
        b = ap.bitcast(bf16)
        shp = " ".join(f"d{i}" for i in range(len(b.shape) - 1))
        return b.rearrange(f"{shp} (f two) -> {shp} f two", two=2)

    with tile.TileContext(nc) as tc:
        with (
            tc.tile_pool(name="const", bufs=1) as cp,
            tc.tile_pool(name="idxp", bufs=2) as ip,
            tc.tile_pool(name="gbuf", bufs=GBUFS) as gp,
            tc.tile_pool(name="mbuf", bufs=2) as mp,
            tc.tile_pool(name="sT", bufs=1) as sp,
            tc.tile_pool(name="rows", bufs=1) as rp,
            tc.tile_pool(name="small", bufs=2) as qp,
            tc.tile_pool(name="spsum", bufs=2, space="PSUM") as pp,
            tc.tile_pool(name="dpsum", bufs=1, space="PSUM") as dp_pool,
            tc.tile_pool(name="tpsum", bufs=2, space="PSUM") as tp,
        ):
            dcol = cp.tile([128, NTILE_TOT], bf16)
            nc.sync.dma_start(out=dcol[:], in_=dcol_in[:])
            iota = cp.tile([128, MWIN], bf16)
            nc.sync.dma_start(out=iota[:], in_=iota_in[:])
            idx_all = cp.tile([128, SLOTS_TOT // 16], i16)
            nc.sync.dma_start(out=idx_all[:], in_=idx_in[:])
            wfl = cp.tile([D, L * 3 * D], bf16)
            nc.sync.dma_start(out=wfl[:], in_=W_in[:])
            bv = cp.tile([D, L], f32)
            nc.sync.dma_start(out=bv[:], in_=b_in[:])
            pwv = cp.tile([D, 1], bf16)
            nc.sync.dma_start(out=pwv[:], in_=pw_in[:])
            pbv = cp.tile([1, 1], f32)
            nc.sync.dma_start(out=pbv[:], in_=pb_in[:])
            normB = cp.tile([D, NCOLS], f32)
            nc.sync.dma_start(out=normB[:], in_=normB_in[:])
            nsqm2 = cp.tile([128, NCOLS // 128], f32)
            nc.sync.dma_start(out=nsqm2[:], in_=nsqm2_in[:])
            nsqp1 = cp.tile([128, NCOLS // 128], f32)
            nc.sync.dma_start(out=nsqp1[:], in_=nsqp1_in[:])
            ident = cp.tile([64, 64], f32)
            make_identity(nc, ident[:])

            p1T = sp.tile([D, NCOLS], f32, tag="p1T")
            if mode in ("full", "noag", "fused_only"):
                h1T = sp.tile([D, NCOLS], f32, tag="h1T")

            NCALL = -(-NTILE_TOT // GMAX_T)
            NGT = 8   # gather-buffer ring: NGT tags x GBUFS bufs in flight

            def gather_state(table, tag):
                return {"table": table, "next": 0, "tiles": {}, "tag": tag}

            def ensure_calls(st, upto_tile):
                """Issue global 1024-slot gather calls until the tile range
                [0, upto_tile) is covered."""
                while st["next"] * GMAX_T < upto_tile:
                    k = st["next"]
                    t0 = k * GMAX_T
                    tn = min(GMAX_T, NTILE_TOT - t0)
                    gt = gp.tile([128, tn, 2 * D], bf16, tag=f"gc{k % NGT}",
                                 name=f"g_{st['tag']}_{k}")
                    st["tiles"][k] = gt
                    nc.gpsimd.dma_gather(
                        gt[:], st["table"][:, :],
                        idx_all[:, t0 * 8:t0 * 8 + tn * 8],
                        tn * 128, tn * 128, 2 * D,
                        queue_num=k % NQUEUES,
                    )
                    st["next"] += 1

            def spmm_chunk(st, c, tag):
                """Gathers + mask gen + PE reduce for chunk c. Returns psum
                tile [64, CCOLS] (caller evacuates / consumes)."""
                ntc = int(ntile_chunk[c])
                cto = int(chunk_tile_off[c])
                ensure_calls(st, cto + ntc)
                mask = mp.tile([128, ntc * MWIN], bf16, tag="mask",
                               name=f"mask_{tag}_{c}")
                m3 = mask[:].rearrange("p (t o) -> p t o", o=MWIN)
                i3 = iota[:].rearrange("p (o t) -> p o t", o=1).to_broadcast(
                    [128, ntc, MWIN])
                d3 = dcol[:, cto:cto + ntc].rearrange(
                    "p (t o) -> p t o", o=1).to_broadcast([128, ntc, MWIN])
                nc.vector.tensor_tensor(out=m3, in0=i3, in1=d3,
                                        op=mybir.AluOpType.is_equal)
                ps = pp.tile([64, CCOLS], f32, tag="spsum", name=f"ps_{tag}_{c}")
                for w in range(CHW):
                    runs = []
                    for h in (0, 1):   # h = src parity of the group's slots
                        base = int(tile_base[c, h, w])
                        for j in range(int(kc[c, w, h])):
                            runs.append((h, base + j))
                    for r, (h, tglob) in enumerate(runs):
                        tg = cto + tglob
                        gt = st["tiles"][tg // GMAX_T]
                        nc.tensor.matmul(
                            ps[:, MWIN * w:MWIN * (w + 1)],
                            gt[:, tg % GMAX_T, h * D:(h + 1) * D],
                            mask[:, tglob * MWIN:(tglob + 1) * MWIN],
                            start=(r == 0), stop=(r == len(runs) - 1),
                        )
                return ps

            def allgather(nm, g):
                agins, tbl = ag[nm]
                nc.gpsimd.collective_compute(
                    "AllGather",
                    mybir.AluOpType.bypass,
                    ins=[agins[g].ap().opt()],
                    outs=[tbl.ap()[GOFF2[g]:GOFF2[g]
                                   + NCORES * CGRP[g] * (CCOLS // 2),
                                   :].opt()],
                    replica_groups=[list(range(NCORES))],
                )

            def evac_group(rows, nm, g, do_ag):
                """DMA chunk-group g's paired bf16 rows to its agin buffer
                and fire the AllGather piece."""
                agins, _ = ag[nm]
                k0, k1 = CSTART[g] * (CCOLS // 128), CEND[g] * (CCOLS // 128)
                # agin rows are [pair, 2*D]: pair (kb*64+q) holds positions
                # (kb*128+q, kb*128+64+q) in its lo/hi D-halves.
                av = agins[g].ap().rearrange("(k r) f -> r k f", r=64)
                nc.sync.dma_start(out=av[:, :, 0:D], in_=rows[0:64, k0:k1, :])
                nc.sync.dma_start(out=av[:, :, D:2 * D],
                                  in_=rows[64:128, k0:k1, :])
                if do_ag:
                    allgather(nm, g)

            def spmm(table, out_sT, tag, scale, nm, do_ag):
                """SpMM writing transposed result to out_sT and (scaled)
                paired bf16 rows to the allgather input buffers."""
                st = gather_state(table, tag)
                rows = rp.tile([128, NCOLS // 128, D], bf16, tag="rows",
                               name=f"rows_{tag}")
                for c in range(NCH):
                    ps = spmm_chunk(st, c, tag)
                    nc.vector.tensor_copy(
                        out=out_sT[:, c * CCOLS:(c + 1) * CCOLS], in_=ps[:])
                    for kb in range(CCOLS // 128):
                        k = c * (CCOLS // 128) + kb
                        tps = tp.tile([128, D], f32, tag="tpsum",
                                      name=f"tps_{tag}_{k}")
                        nc.tensor.transpose(
                            tps[:], out_sT[:, k * 128:(k + 1) * 128], ident[:])
                        nc.vector.tensor_scalar(
                            out=rows[:, k, :], in0=tps[:],
                            scalar1=scale[:, k:k + 1], scalar2=None,
                            op0=mybir.AluOpType.mult)
                    if nm is not None and c + 1 in CEND:
                        evac_group(rows, nm, CEND.index(c + 1), do_ag)

            def spmm_fused_dense(table, l, t0T, p1T_, outT, tag,
                                 nm=None, do_ag=False, scale=None, head=False):
                """SpMM for P2 fused with the dense layer; optionally also
                writes scale*h rows to the agin buffers (layer-1 h -> table3),
                and the prediction head (layer 2)."""
                p1v = bfview(p1T_[:])
                if DENSE_CONTIG:
                    dummy = qp.tile([64, CCOLS], bf16, tag="dummy", name="dumm")
                st = gather_state(table, tag)
                if nm is not None:
                    rows = rp.tile([128, NCOLS // 128, D], bf16, tag="rows",
                                   name=f"rows_{tag}")
                for c in range(NCH):
                    ps = spmm_chunk(st, c, tag)
                    cc = slice(c * CCOLS, (c + 1) * CCOLS)
                    if outT is None:
                        hout = qp.tile([64, CCOLS], bf16, tag="h2c",
                                       name=f"h2c_{tag}_{c}")
                        occ = slice(0, CCOLS)
                    else:
                        hout = outT
                        occ = cc
                    p2c = qp.tile([64, CCOLS], f32, tag="p2c", name=f"p2c_{tag}_{c}")
                    nc.vector.tensor_copy(out=p2c[:], in_=ps[:])
                    # scaled group: W1' P1 + W2' P2
                    dps = dp_pool.tile([64, CCOLS], f32, tag="dps",
                                       name=f"dps_{tag}_{c}")
                    p1r = dummy[:] if DENSE_CONTIG else p1v[:, cc, 1]
                    p2r = dummy[:] if DENSE_CONTIG else bfview(p2c[:])[:, :, 1]
                    nc.tensor.matmul(
                        dps[:], wfl[:, (l * 3 + 1) * D:(l * 3 + 2) * D],
                        p1r, start=True, stop=False)
                    nc.tensor.matmul(
                        dps[:], wfl[:, (l * 3 + 2) * D:(l * 3 + 3) * D],
                        p2r, start=False, stop=True)
                    # plain group: (W0-W2) T0
                    if t0T is None:   # layer 1: stream x^T chunk from DRAM
                        t0c = qp.tile([64, CCOLS], bf16, tag="t0c",
                                      name=f"t0c_{tag}_{c}")
                        nc.sync.dma_start(out=t0c[:], in_=f0T_in[:, cc])
                        t0v = t0c[:]
                    else:
                        t0v = (dummy[:] if DENSE_CONTIG
                               else bfview(t0T[:, cc])[:, :, 1])
                    dpp = dp_pool.tile([64, CCOLS], f32, tag="dpp",
                                       name=f"dpp_{tag}_{c}")
                    nc.tensor.matmul(
                        dpp[:], wfl[:, (l * 3 + 0) * D:(l * 3 + 1) * D],
                        t0v, start=True, stop=True)
                    tmp = qp.tile([64, CCOLS], f32, tag="tmp", name=f"tmp_{tag}_{c}")
                    nc.vector.tensor_tensor(out=tmp[:], in0=dps[:],
                                            in1=normB[:, cc],
                                            op=mybir.AluOpType.mult)
                    hpre = qp.tile([64, CCOLS], f32, tag="hpre",
                                   name=f"hpre_{tag}_{c}")
                    nc.vector.tensor_tensor(out=hpre[:], in0=tmp[:], in1=dpp[:],
                                            op=mybir.AluOpType.add)
                    nc.scalar.activation(
                        out=hout[:, occ], in_=hpre[:],
                        func=mybir.ActivationFunctionType.Relu,
                        bias=bv[:, l:l + 1], scale=1.0)
                    if nm is not None:
                        for kb in range(CCOLS // 128):
                            k = c * (CCOLS // 128) + kb
                            tps = tp.tile([128, D], f32, tag="tpsum",
                                          name=f"tps_{tag}_{k}")
                            nc.tensor.transpose(
                                tps[:],
                                outT[:, c * CCOLS + kb * 128:
                                     c * CCOLS + (kb + 1) * 128],
                                ident[:])
                            nc.vector.tensor_scalar(
                                out=rows[:, k, :], in0=tps[:],
                                scalar1=scale[:, k:k + 1], scalar2=None,
                                op0=mybir.AluOpType.mult)
                        if c + 1 in CEND:
                            evac_group(rows, nm, CEND.index(c + 1), do_ag)
                    if head:
                        hp = tp.tile([1, CCOLS], f32, tag="hpsum",
                                     name=f"hp_{c}")
                        nc.tensor.matmul(hp[:], pwv[:], hout[:, occ],
                                         start=True, stop=True)
                        yc = ip.tile([1, CCOLS], f32, tag="yc", name=f"yc_{c}")
                        nc.vector.tensor_scalar(
                            out=yc[:1, :], in0=hp[:], scalar1=pbv[:1, :1],
                            scalar2=None, op0=mybir.AluOpType.add)
                        nc.sync.dma_start(
                            out=y_out[c * CCOLS:(c + 1) * CCOLS, :],
                            in_=yc[:1, :])

            if mode == "fused_only":
                nc.vector.tensor_copy(out=p1T[:], in_=normB[:])
                for r in range(repeat):
                    spmm_fused_dense(feat_tbl, 0, None, p1T, h1T, f"r{r}f",
                                     nm="t3", do_ag=False, scale=nsqp1)
                nc.sync.dma_start(out=y_out[:, :], in_=h1T[:1, :NCOLS])
                repeat = 0
            elif mode == "spmm_only":
                for r in range(repeat):
                    spmm(feat_tbl, p1T, f"r{r}s1", nsqm2, "t2", False)
                nc.sync.dma_start(out=y_out[:, :], in_=p1T[:1, :NCOLS])
                repeat = 0
            elif mode == "gather_only":
                for r in range(repeat):
                    st = gather_state(feat_tbl, f"r{r}")
                    for c in range(NCH):
                        ps = spmm_chunk(st, c, f"r{r}")
                        nc.vector.tensor_copy(
                            out=p1T[:, c * CCOLS:(c + 1) * CCOLS], in_=ps[:])
                nc.sync.dma_start(out=y_out[:, :], in_=p1T[:1, :NCOLS])
                repeat = 0
            elif mode == "dma_only":
                # gathers only: one dummy matmul per call consumes one tile
                for r in range(repeat):
                    st = gather_state(feat_tbl, f"r{r}")
                    for k in range(NCALL):
                        ensure_calls(st, k * GMAX_T + 1)
                        gt = st["tiles"][k]
                        ps = pp.tile([64, CCOLS], f32, tag="spsum",
                                     name=f"psd_r{r}_{k}")
                        nc.tensor.matmul(
                            ps[:, 0:MWIN], gt[:, 0, 0:D], iota[:],
                            start=True, stop=True)
                nc.sync.dma_start(out=y_out[:, :], in_=normB[:1, :NCOLS])
                repeat = 0
            do_ag = mode != "noag"
            for r in range(repeat):
                # ---- layer 1 ----
                spmm(feat_tbl, p1T, f"r{r}s1", nsqm2, "t2", do_ag)
                t2 = ag["t2"][1] if do_ag else feat_tbl
                spmm_fused_dense(t2, 0, None, p1T, h1T, f"r{r}s2",
                                 nm="t3", do_ag=do_ag, scale=nsqp1)
                t3 = ag["t3"][1] if do_ag else feat_tbl
                # ---- layer 2 ----
                spmm(t3, p1T, f"r{r}s3", nsqm2, "t4", do_ag)
                t4 = ag["t4"][1] if do_ag else feat_tbl
                spmm_fused_dense(t4, 1, h1T, p1T, None, f"r{r}s4",
                                 head=True)

    _finalize_with_split(nc)
    return nc


def _meta_key(meta):
    return (meta["kc"].tobytes(), meta["NTILE_TOT"])


def _get_runner(meta):
    key = _meta_key(meta)
    if _CACHE.get("key") == key:
        return _CACHE["runner"]
    nc = _build_nc(meta)
    _CACHE["runner"] = _build_runner(nc, NCORES)
    _CACHE["key"] = key
    return _CACHE["runner"]


def kernel(features, src, dst, W, b, pw, pb):
    in_maps, meta, perms = _host_prep(features, src, dst, W, b, pw, pb)
    run = _get_runner(meta)
    results, times = run(in_maps, iters=1)
    _CACHE["last_times"] = times
    y = np.empty((N, 1), dtype=np.float32)
    for i in range(NCORES):
        y[i * NSH:(i + 1) * NSH, 0] = results[i]["y"][perms[i], 0]
    return y


# revision 52
# speedup vs baseline: 1.0678x; 1.0678x over previous
"""ChebNet (K=3, L=2) forward on 8 Trainium2 NeuronCores.

Node-sharded SPMD, compiled per graph instance. Each core owns 6250 dst rows,
LPT-permuted into 104 windows of 64 columns so window edge counts are nearly
equal; the shared program uses max-over-cores tile counts per (chunk, half,
window) group (~10% slot padding vs 60% for a fixed layout).

Gather tables are bf16 with two node rows packed per 256B table row (the
dma_gather minimum element); edge slots are grouped by (window, src parity)
so each 128-slot tile reads one half of its gathered elements as the PE
lhsT — no separate parity masks. SpMM = one dma_gather per (chunk, parity)
+ bf16 PE matmuls against 0/1 masks generated in bf16. norm[src] is folded
into the gather tables, norm[dst] into a per-column multiply of the dense
accumulation, which commutes through the weight matmul:

  P1 = A(Nx); T1 = -N P1; table2 = -N^2 P1; P2 = A table2; T2 = -2N P2 - T0
  out^T = (W0-W2)^T T0^T + N o [(-W1)^T P1^T + (-2 W2)^T P2^T]; h = relu(out+b)

Full node tables are rebuilt between SpMMs with AllGather. The table rows are
laid out [chunk-group][core][rows] so each AllGather is split into NG pieces,
each fired as soon as its chunk-group's rows are evacuated — the collective
pipeline overlaps the producing SpMM's remaining compute.
"""
import math
import numpy as np

N, E, D, K, L = 50000, 800000, 64, 3, 2
NCORES = 8
NSH = N // NCORES              # 6250 dst rows per core
MWIN = 64                      # mask window: dst cols per mask tile
CHW = 8                        # windows per chunk
CCOLS = MWIN * CHW             # 512 cols per chunk (one PSUM bank)
NCH = 13                       # chunks per core
NCOLS = NCH * CCOLS            # 6656 padded cols per core
NW = NCH * CHW                 # 104 windows per core
ROWS_PAD = NCOLS               # padded rows per core block in the table
TBL_ROWS = NCORES * ROWS_PAD   # 53248
TBLP = TBL_ROWS // 2           # 26624 paired bf16 rows (< 32768: int16)
import os as _os
GMAX_T = int(_os.environ.get("K_GMAX_T", "8"))  # max tiles per dma_gather
# (>8 tiles = >1024 idxs per call overruns the SWDGE ring and hangs the HW)
NQUEUES = int(_os.environ.get("K_NQUEUES", "4"))
GBUFS = int(_os.environ.get("K_GBUFS", "3"))
MBUFS = int(_os.environ.get("K_MBUFS", "3"))
DENSE_CONTIG = _os.environ.get("K_DENSE_CONTIG", "0") == "1"

# AllGather pipeline: chunk-groups; piece g fires after its chunks evacuate
CGRP = [5, 5, 3]
NG = len(CGRP)
CSTART = [sum(CGRP[:g]) for g in range(NG)]          # first chunk of group
CEND = [sum(CGRP[:g + 1]) for g in range(NG)]        # one-past-last chunk
GRP_OF = sum(([g] * CGRP[g] for g in range(NG)), [])
GOFF = [NCORES * CCOLS * CSTART[g] for g in range(NG)]  # table row offset

_CACHE = {}


# ---------------------------------------------------------------------------
# Workaround for this walrus build: any instruction carrying >1 sync wait is
# rejected ("Too many sync wait commands"). Hoist extras onto 1-wait NoOps on
# the same engine (per-engine program order preserves semantics).
_ws_counter = [0]


def _split_multiwaits(nc):
    import concourse.mybir as mybir
    n_split = 0
    for fn in nc.m.functions:
        for bb in fn.blocks:
            new_list = []
            changed = False
            for inst in bb.instructions:
                si = inst.sync_info
                waits = list(si.on_wait) if si is not None else []
                if len(waits) > 1:
                    changed = True
                    for w in waits[:-1]:
                        _ws_counter[0] += 1
                        nop = mybir.InstNoOp(
                            name=f"waitsplit-{_ws_counter[0]}",
                            ins=[], outs=[],
                            sync_info=mybir.SyncInfo(on_wait=[w], on_update=[]),
                        )
                        nop.engine = inst.engine
                        nc.register_instruction(nop, overwrite=True)
                        new_list.append(nop)
                        n_split += 1
                    si.on_wait = waits[-1:]
                new_list.append(inst)
            if changed:
                bb.instructions[:] = new_list
    return n_split


def _finalize_with_split(nc):
    import concourse.bass as _bass
    nc.compile()           # Bacc passes (incl. library-load insertion)
    _split_multiwaits(nc)  # after replace_nops_with_events, before freeze
    _bass.Bass.finalize(nc)


def _build_runner(nc, n_cores):
    """SPMD runner over the axon PJRT backend (keeps the jitted executable
    and device-resident inputs so repeat calls can be timed)."""
    import jax
    from jax.sharding import Mesh, PartitionSpec
    from jax.experimental.shard_map import shard_map
    import concourse.mybir as mybir
    from concourse.bass2jax import (
        _bass_exec_p, install_neuronx_cc_hook, partition_id_tensor)

    install_neuronx_cc_hook()
    partition_name = nc.partition_id_tensor.name if nc.partition_id_tensor else None

    in_names, out_names, out_avals, zero_outs = [], [], [], []
    for alloc in nc.m.functions[0].allocations:
        if not isinstance(alloc, mybir.MemoryLocationSet):
            continue
        name = alloc.memorylocations[0].name
        if alloc.kind == "ExternalInput":
            if name != partition_name:
                in_names.append(name)
        elif alloc.kind == "ExternalOutput":
            shape = tuple(alloc.tensor_shape)
            dtype = mybir.dt.np(alloc.dtype)
            out_names.append(name)
            out_avals.append(jax.core.ShapedArray(shape, dtype))
            zero_outs.append(np.zeros(shape, dtype))
    n_params = len(in_names)
    all_in_names = list(in_names) + list(out_names)
    if partition_name is not None:
        all_in_names.append(partition_name)

    def _body(*args):
        operands = list(args)
        if partition_name is not None:
            operands.append(partition_id_tensor())
        outs = _bass_exec_p.bind(
            *operands,
            out_avals=tuple(out_avals),
            in_names=tuple(all_in_names),
            out_names=tuple(out_names),
            lowering_input_output_aliases=(),
            sim_require_finite=True,
            sim_require_nnan=True,
            nc=nc,
        )
        return tuple(outs)

    devices = jax.devices()[:n_cores]
    mesh = Mesh(np.asarray(devices), ("core",))
    in_specs = (PartitionSpec("core"),) * (n_params + len(out_names))
    out_specs = (PartitionSpec("core"),) * len(out_names)
    sharded = jax.jit(
        shard_map(_body, mesh=mesh, in_specs=in_specs, out_specs=out_specs,
                  check_rep=False),
        keep_unused=True,
    )

    def run(in_maps, iters=1):
        import time as _time
        per_core = [[np.asarray(m[name]) for name in in_names] for m in in_maps]
        concat_in = [
            np.concatenate([per_core[c][i] for c in range(n_cores)], axis=0)
            for i in range(n_params)
        ]
        concat_zeros = [
            np.zeros((n_cores * z.shape[0], *z.shape[1:]), z.dtype)
            for z in zero_outs
        ]
        sharding = jax.sharding.NamedSharding(mesh, PartitionSpec("core"))
        dev_in = [jax.device_put(a, sharding) for a in concat_in + concat_zeros]
        out = sharded(*dev_in)
        jax.block_until_ready(out)
        times = []
        for _ in range(iters):
            t0 = _time.perf_counter()
            out = sharded(*dev_in)
            jax.block_until_ready(out)
            times.append(_time.perf_counter() - t0)
        results = [
            {name: np.asarray(out[i]).reshape(n_cores, *out_avals[i].shape)[c]
             for i, name in enumerate(out_names)}
            for c in range(n_cores)
        ]
        return results, times

    return run


def _lpt_windows(deg_local):
    """Assign each of the core's nodes to one of NW windows (<=64 nodes each)
    balancing total degree (LPT greedy). Returns pos[node] in [0, NCOLS)."""
    import heapq
    n = deg_local.shape[0]
    order = np.argsort(-deg_local, kind="stable")
    heap = [(0.0, w) for w in range(NW)]
    heapq.heapify(heap)
    counts = np.zeros(NW, dtype=np.int64)
    pos = np.empty(n, dtype=np.int64)
    for node in order:
        s, w = heapq.heappop(heap)
        pos[node] = (w // CHW) * CCOLS + (w % CHW) * MWIN + counts[w]
        counts[w] += 1
        s += deg_local[node]
        if counts[w] < MWIN:
            heapq.heappush(heap, (s, w))
    return pos


GOFF2 = [NCORES * CSTART[g] * (CCOLS // 2) for g in range(NG)]


def _tbl_row(core, pos):
    """(pair row, parity) for (core, permuted col pos): group-major layout,
    pairing positions (128k+q, 128k+64+q) into one 128-wide bf16 table row."""
    c = pos // CCOLS
    g = np.asarray(GRP_OF, dtype=np.int64)[c]
    goff = np.asarray(GOFF2, dtype=np.int64)[g]
    cstart = np.asarray(CSTART, dtype=np.int64)[g]
    p_loc = pos - cstart * CCOLS          # position within the group's chunks
    kb = p_loc // 128
    q = p_loc % 128
    pair = goff + core * (np.asarray(CGRP, np.int64)[g] * (CCOLS // 2)) \
        + kb * 64 + q % 64
    return pair, q // 64


def _host_prep(features, src, dst, W, b, pw, pb):
    import concourse.mybir as mybir
    bf16 = mybir.dt.np(mybir.dt.bfloat16)

    src = np.asarray(src).astype(np.int64)
    dst = np.asarray(dst).astype(np.int64)
    features = np.asarray(features, dtype=np.float32)
    W = np.asarray(W, dtype=np.float32)
    b = np.asarray(b, dtype=np.float32)
    pw = np.asarray(pw, dtype=np.float32).reshape(D, 1)
    pb = np.asarray(pb, dtype=np.float32).reshape(1)

    deg = np.bincount(dst, minlength=N).astype(np.float32)
    norm = np.clip(deg, 1.0, None) ** -0.5

    core_of_dst = dst // NSH
    pos_local = np.empty(N, dtype=np.int64)   # permuted col within core
    for i in range(NCORES):
        nodes = np.arange(i * NSH, (i + 1) * NSH)
        pos_local[nodes] = _lpt_windows(deg[nodes])
    gpair, gpar = _tbl_row(np.arange(N) // NSH, pos_local)  # paired table row

    # per-edge quantities
    e_core = core_of_dst
    e_pos = pos_local[dst]                   # dst col within owner core
    e_win = e_pos // MWIN                    # global window 0..103
    e_dcol = (e_pos % MWIN).astype(np.float32)
    e_srow = gpair[src]                      # src pair row in packed table
    e_half = gpar[src]                       # src parity within the pair

    # group counts per (core, win, parity) -> shared tile counts k = max
    gk = e_core * (NW * 2) + e_win * 2 + e_half
    cnt = np.bincount(gk, minlength=NCORES * NW * 2).reshape(NCORES, NW, 2)
    kk = -(-cnt.max(axis=0) // 128)          # [NW, 2] tiles per (win, half)
    kk = np.maximum(kk, 1)

    # slot/tile layout shared by all cores.
    # chunk c: call h=0 -> windows 0..7 tiles, then call h=1.
    kc = kk.reshape(NCH, CHW, 2)
    call_tiles = kc.sum(axis=1)              # [NCH, 2] tiles per call
    call_slots = call_tiles * 128
    ntile_chunk = call_tiles.sum(axis=1)     # [NCH]
    NTILE_TOT = int(ntile_chunk.sum())
    SLOTS_TOT = NTILE_TOT * 128
    # tile base within chunk for (h, w)
    tile_base = np.zeros((NCH, 2, CHW), dtype=np.int64)
    for c in range(NCH):
        t = 0
        for h in (0, 1):
            for w in range(CHW):
                tile_base[c, h, w] = t
                t += kc[c, w, h]
    chunk_tile_off = np.concatenate([[0], np.cumsum(ntile_chunk)])[:-1]
    chunk_slot_off = chunk_tile_off * 128
    call_slot_off = np.zeros((NCH, 2), dtype=np.int64)
    for c in range(NCH):
        call_slot_off[c, 0] = chunk_slot_off[c]
        call_slot_off[c, 1] = chunk_slot_off[c] + call_slots[c, 0]

    meta = {
        "kc": kc, "call_tiles": call_tiles, "call_slots": call_slots,
        "ntile_chunk": ntile_chunk, "NTILE_TOT": NTILE_TOT,
        "SLOTS_TOT": SLOTS_TOT, "tile_base": tile_base,
        "chunk_tile_off": chunk_tile_off,
    }

    # shared scaled feature table (bf16, two node rows packed per table row)
    feat_scaled = np.zeros((TBLP, 2, D), dtype=bf16)
    feat_scaled[gpair, gpar] = (features * norm[:, None]).astype(bf16)
    feat_scaled = feat_scaled.reshape(TBLP, 2 * D)

    Wflat = np.zeros((D, L * 3 * D), dtype=np.float32)
    for l in range(L):
        for t, Wt in enumerate((W[l, 0] - W[l, 2], -W[l, 1], -2.0 * W[l, 2])):
            Wflat[:, (l * 3 + t) * D:(l * 3 + t + 1) * D] = Wt
    Wflat = Wflat.astype(bf16)

    iota = np.tile(np.arange(MWIN, dtype=np.float32)[None, :],
                   (128, 1)).astype(bf16)

    in_maps = []
    perms = []
    for i in range(NCORES):
        sel = e_core == i
        s_srow = e_srow[sel]
        s_half = e_half[sel]
        s_win = e_win[sel]
        s_dcol = e_dcol[sel]

        # rank within (win, half) group
        key = s_win * 2 + s_half
        order = np.argsort(key, kind="stable")
        ks = key[order]
        grp_start = np.searchsorted(ks, np.arange(NW * 2), side="left")
        rank = np.arange(ks.size) - grp_start[ks]

        ww = ks // 2
        hh = ks % 2
        c_ = ww // CHW
        wl = ww % CHW
        mt = chunk_tile_off[c_] + tile_base[c_, hh, wl] + rank // 128
        slot = mt * 128 + rank % 128

        idx_slots = np.zeros(SLOTS_TOT, dtype=np.int16)
        dcol_slots = np.full(SLOTS_TOT, -1.0, dtype=np.float32)
        idx_slots[slot] = s_srow[order].astype(np.int16)
        dcol_slots[slot] = s_dcol[order]

        # idx wrap per gather call: global 1024-slot windows (call k covers
        # tiles [8k, 8k+8)); within a call, slot i -> [i%16, i//16]
        idx_arr = np.zeros((16, SLOTS_TOT // 16), dtype=np.int16)
        for k in range(-(-NTILE_TOT // GMAX_T)):
            s0 = k * GMAX_T * 128
            n_ = min(GMAX_T * 128, SLOTS_TOT - s0)
            wrap = idx_slots[s0:s0 + n_].reshape(n_ // 16, 16).T
            idx_arr[:, s0 // 16:(s0 + n_) // 16] = wrap
        idx_arr = np.tile(idx_arr, (8, 1))

        dcol = dcol_slots.reshape(NTILE_TOT, 128).T.astype(bf16)

        # local node data in permuted order
        pos = pos_local[i * NSH:(i + 1) * NSH]
        perms.append(pos)
        nloc = norm[i * NSH:(i + 1) * NSH]
        f0T = np.zeros((D, NCOLS), dtype=np.float32)
        f0T[:, pos] = features[i * NSH:(i + 1) * NSH].T
        f0T = f0T.astype(bf16)
        normB_row = np.zeros(NCOLS, dtype=np.float32)
        normB_row[pos] = nloc
        normB = np.tile(normB_row[None, :], (D, 1))
        nsq_m2 = np.zeros((128, NCOLS // 128), dtype=np.float32)
        nsq_p1 = np.zeros((128, NCOLS // 128), dtype=np.float32)
        nrm_cols = np.zeros(NCOLS, dtype=np.float32)
        nrm_cols[pos] = nloc
        nsq_m2[:, :] = (-(nrm_cols ** 2)).reshape(NCOLS // 128, 128).T
        nsq_p1[:, :] = nrm_cols.reshape(NCOLS // 128, 128).T

        in_maps.append({
            "feat_tbl": feat_scaled,
            "f0T": f0T,
            "idx_all": idx_arr,
            "dcol": dcol,
            "iota": iota,
            "normB": normB,
            "nsq_m2": nsq_m2,
            "nsq_p1": nsq_p1,
            "Wflat": Wflat,
            "bvec": b.T.copy(),
            "pwv": pw.astype(bf16),
            "pbv": pb.reshape(1, 1),
        })
    return in_maps, meta, perms


def _build_nc(meta, repeat=1, mode="full"):
    import concourse.bacc as bacc
    import concourse.mybir as mybir
    import concourse.tile as tile
    from concourse.masks import make_identity
    f32 = mybir.dt.float32
    bf16 = mybir.dt.bfloat16
    i16 = mybir.dt.int16

    kc = meta["kc"]
    call_tiles = meta["call_tiles"]
    call_slots = meta["call_slots"]
    ntile_chunk = meta["ntile_chunk"]
    NTILE_TOT = meta["NTILE_TOT"]
    SLOTS_TOT = meta["SLOTS_TOT"]
    tile_base = meta["tile_base"]
    chunk_tile_off = meta["chunk_tile_off"]

    nc = bacc.Bacc("TRN2", num_swdge_queues=NQUEUES)
    feat_tbl = nc.declare_dram_parameter("feat_tbl", [TBLP, 2 * D], bf16, isOutput=False)
    f0T_in = nc.declare_dram_parameter("f0T", [D, NCOLS], bf16, isOutput=False)
    idx_in = nc.declare_dram_parameter("idx_all", [128, SLOTS_TOT // 16], i16, isOutput=False)
    dcol_in = nc.declare_dram_parameter("dcol", [128, NTILE_TOT], bf16, isOutput=False)
    iota_in = nc.declare_dram_parameter("iota", [128, MWIN], bf16, isOutput=False)
    normB_in = nc.declare_dram_parameter("normB", [D, NCOLS], f32, isOutput=False)
    nsqm2_in = nc.declare_dram_parameter("nsq_m2", [128, NCOLS // 128], f32, isOutput=False)
    nsqp1_in = nc.declare_dram_parameter("nsq_p1", [128, NCOLS // 128], f32, isOutput=False)
    W_in = nc.declare_dram_parameter("Wflat", [D, L * 3 * D], bf16, isOutput=False)
    b_in = nc.declare_dram_parameter("bvec", [D, L], f32, isOutput=False)
    pw_in = nc.declare_dram_parameter("pwv", [D, 1], bf16, isOutput=False)
    pb_in = nc.declare_dram_parameter("pbv", [1, 1], f32, isOutput=False)
    y_out = nc.declare_dram_parameter("y", [NCOLS, 1], f32, isOutput=True)

    ag = {}
    for nm in ("t2", "t3", "t4"):
        agins = [nc.dram_tensor(f"agin_{nm}_{g}",
                                [CGRP[g] * (CCOLS // 2), 2 * D], bf16)
                 for g in range(NG)]
        tbl = nc.dram_tensor(f"tbl_{nm}", [TBLP, 2 * D], bf16,
                             addr_space="Shared")
        ag[nm] = (agins, tbl)

    def bfview(ap):
        """View the high half-words of an f32 AP as a stride-2 bf16 AP."""
        b = ap.bitcast(bf16)
        shp = " ".join(f"d{i}" for i in range(len(b.shape) - 1))
        return b.rearrange(f"{shp} (f two) -> {shp} f two", two=2)

    with tile.TileContext(nc) as tc:
        with (
            tc.tile_pool(name="const", bufs=1) as cp,
            tc.tile_pool(name="idxp", bufs=2) as ip,
            tc.tile_pool(name="gbuf", bufs=GBUFS) as gp,
            tc.tile_pool(name="mbuf", bufs=MBUFS) as mp,
            tc.tile_pool(name="sT", bufs=1) as sp,
            tc.tile_pool(name="rows", bufs=1) as rp,
            tc.tile_pool(name="small", bufs=2) as qp,
            tc.tile_pool(name="spsum", bufs=2, space="PSUM") as pp,
            tc.tile_pool(name="dpsum", bufs=1, space="PSUM") as dp_pool,
            tc.tile_pool(name="tpsum", bufs=2, space="PSUM") as tp,
        ):
            dcol = cp.tile([128, NTILE_TOT], bf16)
            nc.sync.dma_start(out=dcol[:], in_=dcol_in[:])
            iota = cp.tile([128, MWIN], bf16)
            nc.sync.dma_start(out=iota[:], in_=iota_in[:])
            idx_all = cp.tile([128, SLOTS_TOT // 16], i16)
            nc.sync.dma_start(out=idx_all[:], in_=idx_in[:])
            wfl = cp.tile([D, L * 3 * D], bf16)
            nc.sync.dma_start(out=wfl[:], in_=W_in[:])
            bv = cp.tile([D, L], f32)
            nc.sync.dma_start(out=bv[:], in_=b_in[:])
            pwv = cp.tile([D, 1], bf16)
            nc.sync.dma_start(out=pwv[:], in_=pw_in[:])
            pbv = cp.tile([1, 1], f32)
            nc.sync.dma_start(out=pbv[:], in_=pb_in[:])
            normB = cp.tile([D, NCOLS], f32)
            nc.sync.dma_start(out=normB[:], in_=normB_in[:])
            nsqm2 = cp.tile([128, NCOLS // 128], f32)
            nc.sync.dma_start(out=nsqm2[:], in_=nsqm2_in[:])
            nsqp1 = cp.tile([128, NCOLS // 128], f32)
            nc.sync.dma_start(out=nsqp1[:], in_=nsqp1_in[:])
            ident = cp.tile([64, 64], f32)
            make_identity(nc, ident[:])

            p1T = sp.tile([D, NCOLS], f32, tag="p1T")
            if mode in ("full", "noag", "fused_only"):
                h1T = sp.tile([D, NCOLS], f32, tag="h1T")

            NCALL = -(-NTILE_TOT // GMAX_T)
            NGT = 8   # gather-buffer ring: NGT tags x GBUFS bufs in flight

            def gather_state(table, tag):
                return {"table": table, "next": 0, "tiles": {}, "tag": tag}

            def ensure_calls(st, upto_tile):
                """Issue global 1024-slot gather calls until the tile range
                [0, upto_tile) is covered."""
                while st["next"] * GMAX_T < upto_tile:
                    k = st["next"]
                    t0 = k * GMAX_T
                    tn = min(GMAX_T, NTILE_TOT - t0)
                    gt = gp.tile([128, tn, 2 * D], bf16, tag=f"gc{k % NGT}",
                                 name=f"g_{st['tag']}_{k}")
                    st["tiles"][k] = gt
                    nc.gpsimd.dma_gather(
                        gt[:], st["table"][:, :],
                        idx_all[:, t0 * 8:t0 * 8 + tn * 8],
                        tn * 128, tn * 128, 2 * D,
                        queue_num=k % NQUEUES,
                    )
                    st["next"] += 1

            def spmm_chunk(st, c, tag):
                """Gathers + mask gen + PE reduce for chunk c. Returns psum
                tile [64, CCOLS] (caller evacuates / consumes)."""
                ntc = int(ntile_chunk[c])
                cto = int(chunk_tile_off[c])
                ensure_calls(st, cto + ntc)
                mask = mp.tile([128, ntc * MWIN], bf16, tag="mask",
                               name=f"mask_{tag}_{c}")
                m3 = mask[:].rearrange("p (t o) -> p t o", o=MWIN)
                i3 = iota[:].rearrange("p (o t) -> p o t", o=1).to_broadcast(
                    [128, ntc, MWIN])
                d3 = dcol[:, cto:cto + ntc].rearrange(
                    "p (t o) -> p t o", o=1).to_broadcast([128, ntc, MWIN])
                nc.vector.tensor_tensor(out=m3, in0=i3, in1=d3,
                                        op=mybir.AluOpType.is_equal)
                ps = pp.tile([64, CCOLS], f32, tag="spsum", name=f"ps_{tag}_{c}")
                for w in range(CHW):
                    runs = []
                    for h in (0, 1):   # h = src parity of the group's slots
                        base = int(tile_base[c, h, w])
                        for j in range(int(kc[c, w, h])):
                            runs.append((h, base + j))
                    for r, (h, tglob) in enumerate(runs):
                        tg = cto + tglob
                        gt = st["tiles"][tg // GMAX_T]
                        nc.tensor.matmul(
                            ps[:, MWIN * w:MWIN * (w + 1)],
                            gt[:, tg % GMAX_T, h * D:(h + 1) * D],
                            mask[:, tglob * MWIN:(tglob + 1) * MWIN],
                            start=(r == 0), stop=(r == len(runs) - 1),
                        )
                return ps

            def allgather(nm, g):
                agins, tbl = ag[nm]
                nc.gpsimd.collective_compute(
                    "AllGather",
                    mybir.AluOpType.bypass,
                    ins=[agins[g].ap().opt()],
                    outs=[tbl.ap()[GOFF2[g]:GOFF2[g]
                                   + NCORES * CGRP[g] * (CCOLS // 2),
                                   :].opt()],
                    replica_groups=[list(range(NCORES))],
                )

            def evac_group(rows, nm, g, do_ag):
                """DMA chunk-group g's paired bf16 rows to its agin buffer
                and fire the AllGather piece."""
                agins, _ = ag[nm]
                k0, k1 = CSTART[g] * (CCOLS // 128), CEND[g] * (CCOLS // 128)
                # agin rows are [pair, 2*D]: pair (kb*64+q) holds positions
                # (kb*128+q, kb*128+64+q) in its lo/hi D-halves.
                av = agins[g].ap().rearrange("(k r) f -> r k f", r=64)
                nc.sync.dma_start(out=av[:, :, 0:D], in_=rows[0:64, k0:k1, :])
                nc.sync.dma_start(out=av[:, :, D:2 * D],
                                  in_=rows[64:128, k0:k1, :])
                if do_ag:
                    allgather(nm, g)

            def spmm(table, out_sT, tag, scale, nm, do_ag):
                """SpMM writing transposed result to out_sT and (scaled)
                paired bf16 rows to the allgather input buffers."""
                st = gather_state(table, tag)
                rows = rp.tile([128, NCOLS // 128, D], bf16, tag="rows",
                               name=f"rows_{tag}")
                for c in range(NCH):
                    ps = spmm_chunk(st, c, tag)
                    nc.vector.tensor_copy(
                        out=out_sT[:, c * CCOLS:(c + 1) * CCOLS], in_=ps[:])
                    for kb in range(CCOLS // 128):
                        k = c * (CCOLS // 128) + kb
                        tps = tp.tile([128, D], f32, tag="tpsum",
                                      name=f"tps_{tag}_{k}")
                        nc.tensor.transpose(
                            tps[:], out_sT[:, k * 128:(k + 1) * 128], ident[:])
                        nc.vector.tensor_scalar(
                            out=rows[:, k, :], in0=tps[:],
                            scalar1=scale[:, k:k + 1], scalar2=None,
                            op0=mybir.AluOpType.mult)
                    if nm is not None and c + 1 in CEND:
                        evac_group(rows, nm, CEND.index(c + 1), do_ag)

            def spmm_fused_dense(table, l, t0T, p1T_, outT, tag,
                                 nm=None, do_ag=False, scale=None, head=False):
                """SpMM for P2 fused with the dense layer; optionally also
                writes scale*h rows to the agin buffers (layer-1 h -> table3),
                and the prediction head (layer 2)."""
                p1v = bfview(p1T_[:])
                if DENSE_CONTIG:
                    dummy = qp.tile([64, CCOLS], bf16, tag="dummy", name="dumm")
                st = gather_state(table, tag)
                if nm is not None:
                    rows = rp.tile([128, NCOLS // 128, D], bf16, tag="rows",
                                   name=f"rows_{tag}")
                for c in range(NCH):
                    ps = spmm_chunk(st, c, tag)
                    cc = slice(c * CCOLS, (c + 1) * CCOLS)
                    if outT is None:
                        hout = qp.tile([64, CCOLS], bf16, tag="h2c",
                                       name=f"h2c_{tag}_{c}")
                        occ = slice(0, CCOLS)
                    else:
                        hout = outT
                        occ = cc
                    p2c = qp.tile([64, CCOLS], f32, tag="p2c", name=f"p2c_{tag}_{c}")
                    nc.vector.tensor_copy(out=p2c[:], in_=ps[:])
                    # scaled group: W1' P1 + W2' P2
                    dps = dp_pool.tile([64, CCOLS], f32, tag="dps",
                                       name=f"dps_{tag}_{c}")
                    p1r = dummy[:] if DENSE_CONTIG else p1v[:, cc, 1]
                    p2r = dummy[:] if DENSE_CONTIG else bfview(p2c[:])[:, :, 1]
                    nc.tensor.matmul(
                        dps[:], wfl[:, (l * 3 + 1) * D:(l * 3 + 2) * D],
                        p1r, start=True, stop=False)
                    nc.tensor.matmul(
                        dps[:], wfl[:, (l * 3 + 2) * D:(l * 3 + 3) * D],
                        p2r, start=False, stop=True)
                    # plain group: (W0-W2) T0
                    if t0T is None:   # layer 1: stream x^T chunk from DRAM
                        t0c = qp.tile([64, CCOLS], bf16, tag="t0c",
                                      name=f"t0c_{tag}_{c}")
                        nc.sync.dma_start(out=t0c[:], in_=f0T_in[:, cc])
                        t0v = t0c[:]
                    else:
                        t0v = (dummy[:] if DENSE_CONTIG
                               else bfview(t0T[:, cc])[:, :, 1])
                    dpp = dp_pool.tile([64, CCOLS], f32, tag="dpp",
                                       name=f"dpp_{tag}_{c}")
                    nc.tensor.matmul(
                        dpp[:], wfl[:, (l * 3 + 0) * D:(l * 3 + 1) * D],
                        t0v, start=True, stop=True)
                    tmp = qp.tile([64, CCOLS], f32, tag="tmp", name=f"tmp_{tag}_{c}")
                    nc.vector.tensor_tensor(out=tmp[:], in0=dps[:],
                                            in1=normB[:, cc],
                                            op=mybir.AluOpType.mult)
                    hpre = qp.tile([64, CCOLS], f32, tag="hpre",
                                   name=f"hpre_{tag}_{c}")
                    nc.vector.tensor_tensor(out=hpre[:], in0=tmp[:], in1=dpp[:],
                                            op=mybir.AluOpType.add)
                    nc.scalar.activation(
                        out=hout[:, occ], in_=hpre[:],
                        func=mybir.ActivationFunctionType.Relu,
                        bias=bv[:, l:l + 1], scale=1.0)
                    if nm is not None:
                        for kb in range(CCOLS // 128):
                            k = c * (CCOLS // 128) + kb
                            tps = tp.tile([128, D], f32, tag="tpsum",
                                          name=f"tps_{tag}_{k}")
                            nc.tensor.transpose(
                                tps[:],
                                outT[:, c * CCOLS + kb * 128:
                                     c * CCOLS + (kb + 1) * 128],
                                ident[:])
                            nc.vector.tensor_scalar(
                                out=rows[:, k, :], in0=tps[:],
                                scalar1=scale[:, k:k + 1], scalar2=None,
                                op0=mybir.AluOpType.mult)
                        if c + 1 in CEND:
                            evac_group(rows, nm, CEND.index(c + 1), do_ag)
                    if head:
                        hp = tp.tile([1, CCOLS], f32, tag="hpsum",
                                     name=f"hp_{c}")
                        nc.tensor.matmul(hp[:], pwv[:], hout[:, occ],
                                         start=True, stop=True)
                        yc = ip.tile([1, CCOLS], f32, tag="yc", name=f"yc_{c}")
                        nc.vector.tensor_scalar(
                            out=yc[:1, :], in0=hp[:], scalar1=pbv[:1, :1],
                            scalar2=None, op0=mybir.AluOpType.add)
                        nc.sync.dma_start(
                            out=y_out[c * CCOLS:(c + 1) * CCOLS, :],
                            in_=yc[:1, :])

            if mode == "fused_only":
                nc.vector.tensor_copy(out=p1T[:], in_=normB[:])
                for r in range(repeat):
                    spmm_fused_dense(feat_tbl, 0, None, p1T, h1T, f"r{r}f",
                                     nm="t3", do_ag=False, scale=nsqp1)
                nc.sync.dma_start(out=y_out[:, :], in_=h1T[:1, :NCOLS])
                repeat = 0
            elif mode == "spmm_only":
                for r in range(repeat):
                    spmm(feat_tbl, p1T, f"r{r}s1", nsqm2, "t2", False)
                nc.sync.dma_start(out=y_out[:, :], in_=p1T[:1, :NCOLS])
                repeat = 0
            elif mode == "gather_only":
                for r in range(repeat):
                    st = gather_state(feat_tbl, f"r{r}")
                    for c in range(NCH):
                        ps = spmm_chunk(st, c, f"r{r}")
                        nc.vector.tensor_copy(
                            out=p1T[:, c * CCOLS:(c + 1) * CCOLS], in_=ps[:])
                nc.sync.dma_start(out=y_out[:, :], in_=p1T[:1, :NCOLS])
                repeat = 0
            elif mode == "dma_only":
                # gathers only: one dummy matmul per call consumes one tile
                for r in range(repeat):
                    st = gather_state(feat_tbl, f"r{r}")
                    for k in range(NCALL):
                        ensure_calls(st, k * GMAX_T + 1)
                        gt = st["tiles"][k]
                        ps = pp.tile([64, CCOLS], f32, tag="spsum",
                                     name=f"psd_r{r}_{k}")
                        nc.tensor.matmul(
                            ps[:, 0:MWIN], gt[:, 0, 0:D], iota[:],
                            start=True, stop=True)
                nc.sync.dma_start(out=y_out[:, :], in_=normB[:1, :NCOLS])
                repeat = 0
            do_ag = mode != "noag"
            for r in range(repeat):
                # ---- layer 1 ----
                spmm(feat_tbl, p1T, f"r{r}s1", nsqm2, "t2", do_ag)
                t2 = ag["t2"][1] if do_ag else feat_tbl
                spmm_fused_dense(t2, 0, None, p1T, h1T, f"r{r}s2",
                                 nm="t3", do_ag=do_ag, scale=nsqp1)
                t3 = ag["t3"][1] if do_ag else feat_tbl
                # ---- layer 2 ----
                spmm(t3, p1T, f"r{r}s3", nsqm2, "t4", do_ag)
                t4 = ag["t4"][1] if do_ag else feat_tbl
                spmm_fused_dense(t4, 1, h1T, p1T, None, f"r{r}s4",
                                 head=True)

    _finalize_with_split(nc)
    return nc


def _meta_key(meta):
    return (meta["kc"].tobytes(), meta["NTILE_TOT"])


def _get_runner(meta):
    key = _meta_key(meta)
    if _CACHE.get("key") == key:
        return _CACHE["runner"]
    nc = _build_nc(meta)
    _CACHE["runner"] = _build_runner(nc, NCORES)
    _CACHE["key"] = key
    return _CACHE["runner"]


def kernel(features, src, dst, W, b, pw, pb):
    in_maps, meta, perms = _host_prep(features, src, dst, W, b, pw, pb)
    run = _get_runner(meta)
    results, times = run(in_maps, iters=1)
    _CACHE["last_times"] = times
    y = np.empty((N, 1), dtype=np.float32)
    for i in range(NCORES):
        y[i * NSH:(i + 1) * NSH, 0] = results[i]["y"][perms[i], 0]
    return y
